# revision 1
# baseline (speedup 1.0000x reference)
"""ALSHConvNet on 8 TRN2 NeuronCores — pure data parallel (batch/8 per core).

Per core (512 samples):
- Convs as fp16 banded matmuls on TensorE, fp32 PSUM accumulation. Output
  pixel chunks packed into M with channels: M-order (parity, xpair, ch) so
  maxpool-x partners are the two contiguous partition halves.
- ALSH hash/mask path in fp32 (PE reductions, DVE/ACT elementwise, rank-2
  XNOR matmul for the [ch x batch] mask map).
- Maxpool: y-pairs = DVE max of adjacent column tiles; x-pairs = SBUF->SBUF
  DMA of the odd partition half + DVE max.
- Host does layout only: sharding, conv1 im2col, banded weight matrices,
  padding, constant selectors.
"""

import sys

for p in ("/opt/trn_rl_repo",):
    if p not in sys.path:
        sys.path.insert(0, p)

import numpy as np

import concourse.bass as bass  # noqa
import concourse.bacc as bacc
import concourse.mybir as mybir
import concourse.tile as tile
from concourse import bass_isa
from concourse.bass_utils import run_bass_kernel_spmd

F32 = mybir.dt.float32
F16 = mybir.dt.float16
AF = mybir.ActivationFunctionType
ALU = mybir.AluOpType
AX = mybir.AxisListType

NCORES = 8
B = 512
R = 0.2
EPS = 1e-12

_CACHED = {}


# ---------------------------------------------------------------- host prep
def _band_lhsT1(W1):
    l0 = np.zeros((108, 128), np.float32)
    l1 = np.zeros((72, 128), np.float32)
    for par in range(2):
        for oxp in range(4):
            for co in range(16):
                m = par * 64 + oxp * 16 + co
                oxl = 2 * oxp + par
                for ky in range(5):
                    for ci in range(3):
                        for kx in range(5):
                            wx = oxl + kx
                            if ky < 3:
                                l0[ky * 36 + ci * 12 + wx, m] = W1[co, ci, ky, kx]
                            else:
                                l1[(ky - 3) * 36 + ci * 12 + wx, m] = W1[co, ci, ky, kx]
    return l0.astype(np.float16), l1.astype(np.float16)


def _band_lhsT2(W2):
    l = np.zeros((5, 128, 80), np.float32)
    for ky in range(5):
        for par in range(2):
            for oxp in range(2):
                for co in range(20):
                    m = par * 40 + oxp * 20 + co
                    oxl = 2 * oxp + par
                    for ci in range(16):
                        for kx in range(5):
                            l[ky, (oxl + kx) * 16 + ci, m] = W2[co, ci, ky, kx]
    return l.astype(np.float16)


def _band_lhsT3(W3):
    l = np.zeros((5, 120, 40), np.float32)
    for ky in range(5):
        for par in range(2):
            for co in range(20):
                m = par * 20 + co
                for ci in range(20):
                    for kx in range(5):
                        l[ky, (par + kx) * 20 + ci, m] = W3[co, ci, ky, kx]
    return l.astype(np.float16)


def _fc_lhsT(Wo):
    l = np.zeros((4, 80, 10), np.float32)
    for d in range(4):
        for oyp in range(4):
            for co in range(20):
                l[d, oyp * 20 + co, :] = Wo[:, co * 16 + oyp * 4 + d]
    return l.astype(np.float16)


def _im2col1(xs):
    xp = np.zeros((B, 3, 36, 36), np.float16)
    xp[:, :, 2:34, 2:34] = xs.astype(np.float16)
    g0 = np.empty((4, 108, 32, B), np.float16)
    g1 = np.empty((4, 72, 32, B), np.float16)
    for c in range(4):
        for ky in range(5):
            blk = xp[:, :, ky : ky + 32, 8 * c : 8 * c + 12].transpose(1, 3, 2, 0)
            if ky < 3:
                g0[c, ky * 36 : (ky + 1) * 36].reshape(3, 12, 32, B)[:] = blk
            else:
                g1[c, (ky - 3) * 36 : (ky - 2) * 36].reshape(3, 12, 32, B)[:] = blk
    return g0, g1


def _morder(nrep_par, nxp, C):
    return [co for _ in range(nrep_par) for _ in range(nxp) for co in range(C)]


def _host_prep(inputs):
    x = inputs["x"].astype(np.float32)
    l10, l11 = _band_lhsT1(inputs["W1"].astype(np.float32))
    b1 = inputs["b1"].astype(np.float32)
    b2 = inputs["b2"].astype(np.float32)
    b3 = inputs["b3"].astype(np.float32)
    shared = {
        "l1g0": l10,
        "l1g1": l11,
        "l2": _band_lhsT2(inputs["W2"].astype(np.float32)),
        "l3": _band_lhsT3(inputs["W3"].astype(np.float32)),
        "lo": _fc_lhsT(inputs["Wo"].astype(np.float32)),
        "s1": np.repeat(np.eye(3, dtype=np.float32), 32, axis=0),
        "s2": np.tile(np.eye(16, dtype=np.float16), (4, 1)),
        "s3": np.tile(np.eye(20, dtype=np.float16), (2, 1)),
        "hw1": inputs["W1"].reshape(16, 75).astype(np.float32),
        "hw2": inputs["W2"].reshape(20, 400).astype(np.float32),
        "hw3": inputs["W3"].reshape(20, 500).astype(np.float32),
        "a1r": inputs["a1"][:75].reshape(3, 25).astype(np.float32),
        "a1t": inputs["a1"][75:].reshape(1, 5).astype(np.float32),
        "a2r": inputs["a2"][:400].reshape(16, 25).astype(np.float32),
        "a2t": inputs["a2"][400:].reshape(1, 5).astype(np.float32),
        "a3r": inputs["a3"][:500].reshape(20, 25).astype(np.float32),
        "a3t": inputs["a3"][500:].reshape(1, 5).astype(np.float32),
        "c1": inputs["c1"].reshape(1, 1).astype(np.float32),
        "c2": inputs["c2"].reshape(1, 1).astype(np.float32),
        "c3": inputs["c3"].reshape(1, 1).astype(np.float32),
        "b1m": b1[np.array(_morder(2, 4, 16))].reshape(-1, 1),
        "b2m": b2[np.array(_morder(2, 2, 20))].reshape(-1, 1),
        "b3m": b3[np.array(_morder(2, 1, 20))].reshape(-1, 1),
        "bo": inputs["bo"].reshape(10, 1).astype(np.float32),
        "ones_row": np.ones((1, 20), np.float32),
        "ones_col": np.ones((20, 1), np.float32),
    }
    in_maps = []
    for i in range(NCORES):
        xs = x[i * B : (i + 1) * B]
        g0, g1 = _im2col1(xs)
        m = dict(shared)
        m["rhs1g0"] = g0
        m["rhs1g1"] = g1
        m["xq"] = np.ascontiguousarray(xs.transpose(1, 2, 0, 3).reshape(96, B * 32))
        in_maps.append(m)
    return in_maps


# ---------------------------------------------------------------- device build
def _parity_ge1(nc, pool, t_ap, C, outtile):
    """outtile = (floor(t) mod 2) as 0/1 via fp32 magic rounding.
    Valid because |t| < 2^21 and t is >=5e-5 away from every integer."""
    MAGIC = 12582912.0  # 1.5 * 2^23
    a = pool.tile([C, t_ap.shape[1]], F32, tag="par_a", name="par_a")
    nc.vector.tensor_scalar(a[:], t_ap, 0.5, -0.5, ALU.mult, ALU.add)
    nc.vector.tensor_scalar_add(a[:], a[:], MAGIC)
    nc.vector.tensor_scalar_add(a[:], a[:], -MAGIC)  # a = floor(t/2)
    u = pool.tile([C, t_ap.shape[1]], F32, tag="par_u", name="par_u")
    nc.vector.scalar_tensor_tensor(u[:], a[:], -2.0, t_ap, ALU.mult, ALU.add)
    nc.vector.tensor_scalar(outtile, u[:], 1.0, None, ALU.is_ge)


def _bcast_row(nc, pool, pspool, row_ap, C, ones_row, tag, dtype=F32):
    """[C, N] tile = broadcast of row_ap [1, N] to C partitions (rank-1 PE)."""
    N = row_ap.shape[1]
    ps = pspool.tile([C, N], F32, tag="accps", name="bc_ps")
    nc.tensor.matmul(ps[:], ones_row[0:1, 0:C], row_ap, start=True, stop=True)
    t = pool.tile([C, N], dtype, tag=tag, name=tag)
    nc.vector.tensor_copy(t[:], ps[:])
    return t


def _hash_bits(nc, pool, pspool, Kf, aflat, atail, cc, C, ones_row):
    """kh [C,1] fp32 from weight matrix tile Kf [C, D]."""
    D = Kf.shape[1]
    sq = pool.tile([C, D], F32, tag="hsq", name="hsq")
    nc.vector.tensor_tensor(sq[:], Kf[:], Kf[:], ALU.mult)
    n2 = pool.tile([C, 1], F32, tag="hn2", name="hn2")
    nc.vector.tensor_reduce(n2[:], sq[:], AX.X, ALU.add)
    nrm = pool.tile([C, 1], F32, tag="hnrm", name="hnrm")
    nc.scalar.activation(nrm[:], n2[:], AF.Sqrt)
    nrow = pool.tile([1, C], F32, tag="hnrow", name="hnrow")
    nc.sync.dma_start(nrow[0:1, :], nrm[:, 0:1])
    nmx = pool.tile([1, 1], F32, tag="hnmx", name="hnmx")
    nc.vector.tensor_reduce(nmx[:], nrow[:], AX.X, ALU.max)
    nc.vector.tensor_scalar_add(nmx[:], nmx[:], EPS)
    rm = pool.tile([1, 1], F32, tag="hrm", name="hrm")
    nc.vector.reciprocal(rm[:], nmx[:])
    rmax = _bcast_row(nc, pool, pspool, rm[0:1, 0:1], C, ones_row, "hrmax")
    n = pool.tile([C, 1], F32, tag="hn", name="hn")
    nc.vector.tensor_tensor(n[:], nrm[:], rmax[:, 0:1], ALU.mult)
    pw = pool.tile([C, 5], F32, tag="hpw", name="hpw")
    nc.vector.tensor_tensor(pw[:, 0:1], n[:], n[:], ALU.mult)
    for i in range(1, 5):
        nc.vector.tensor_tensor(
            pw[:, i : i + 1], pw[:, i - 1 : i], pw[:, i - 1 : i], ALU.mult
        )
    atb = _bcast_row(nc, pool, pspool, atail[0:1, :], C, ones_row, "hatb")
    nc.vector.tensor_tensor(pw[:], pw[:], atb[:], ALU.mult)
    sb = pool.tile([C, 1], F32, tag="hsb", name="hsb")
    nc.vector.tensor_reduce(sb[:], pw[:], AX.X, ALU.add)
    kn = pool.tile([C, D], F32, tag="hkn", name="hkn")
    nc.vector.tensor_scalar(kn[:], Kf[:], rmax[:, 0:1], None, ALU.mult)
    arb = _bcast_row(nc, pool, pspool, aflat[0:1, :], C, ones_row, "harb")
    nc.vector.tensor_tensor(kn[:], kn[:], arb[:], ALU.mult)
    sa = pool.tile([C, 1], F32, tag="hsa", name="hsa")
    nc.vector.tensor_reduce(sa[:], kn[:], AX.X, ALU.add)
    kv = pool.tile([C, 1], F32, tag="hkv", name="hkv")
    nc.vector.tensor_tensor(kv[:], sa[:], sb[:], ALU.add)
    ccb = _bcast_row(nc, pool, pspool, cc[0:1, 0:1], C, ones_row, "hccb")
    nc.vector.tensor_tensor(kv[:], kv[:], ccb[:, 0:1], ALU.add)
    nc.vector.tensor_scalar_mul(kv[:], kv[:], 1.0 / R)
    kh = pool.tile([C, 1], F32, tag="hkh", name="hkh")
    _parity_ge1(nc, pool, kv[:], C, kh[:])
    return kh


def _query_bits(nc, pool, pspool, cms, Av, tail, cc, C, ones_col, tag):
    num_ps = pspool.tile([1, B], F32, tag="accps", name="qnum_ps")
    nc.tensor.matmul(num_ps[:], Av[:, 0:1], cms[:], start=True, stop=True)
    sqt = pool.tile([C, B], F32, tag="q_t", name="q_t")
    nc.vector.tensor_tensor(sqt[:], cms[:], cms[:], ALU.mult)
    s2_ps = pspool.tile([1, B], F32, tag="accps", name="qs2_ps")
    nc.tensor.matmul(s2_ps[:], ones_col[0:C, 0:1], sqt[:], start=True, stop=True)
    den = pool.tile([1, B], F32, tag="q_den", name="q_den")
    nc.scalar.activation(den[0:1, :], s2_ps[0:1, :], AF.Sqrt)
    nc.vector.tensor_scalar_mul(den[0:1, :], den[0:1, :], 5.0)
    nc.vector.tensor_scalar_add(den[0:1, :], den[0:1, :], EPS)
    rden = pool.tile([1, B], F32, tag="q_rden", name="q_rden")
    nc.vector.reciprocal(rden[0:1, :], den[0:1, :])
    qh = pool.tile([1, B], F32, tag=tag, name=tag)
    nc.vector.tensor_tensor(qh[0:1, :], num_ps[0:1, :], rden[0:1, :], ALU.mult)
    nc.vector.tensor_scalar_add(qh[0:1, :], qh[0:1, :], tail[0:1, 0:1])
    nc.vector.tensor_scalar_add(qh[0:1, :], qh[0:1, :], cc[0:1, 0:1])
    nc.vector.tensor_scalar_mul(qh[0:1, :], qh[0:1, :], 1.0 / R)
    _parity_ge1(nc, pool, qh[0:1, :], 1, qh[0:1, :])
    return qh


def _mask_map(nc, pool, pspool, kh, qh, P, tag):
    """[P, B] fp16 = XNOR(kh[ch(p)], qh[b]); p-order cycles channels fastest."""
    C = kh.shape[0]
    khrow = pool.tile([1, 2 * C], F32, tag="mmkhrow", name="mmkhrow")
    nc.sync.dma_start(khrow[0:1, 0:C], kh[:, 0:1])
    nc.vector.tensor_scalar(
        khrow[0:1, C : 2 * C], khrow[0:1, 0:C], -1.0, 1.0, ALU.mult, ALU.add
    )
    lhsT = pool.tile([2, P], F32, tag="mmlhsT", name="mmlhsT")
    for r in range(P // C):
        nc.sync.dma_start(lhsT[0:1, r * C : (r + 1) * C], khrow[0:1, 0:C])
        nc.sync.dma_start(lhsT[1:2, r * C : (r + 1) * C], khrow[0:1, C : 2 * C])
    qrow = pool.tile([1, 2 * B], F32, tag="mmqrow", name="mmqrow")
    nc.vector.tensor_copy(qrow[0:1, 0:B], qh[0:1, :])
    nc.vector.tensor_scalar(
        qrow[0:1, B : 2 * B], qh[0:1, :], -1.0, 1.0, ALU.mult, ALU.add
    )
    rhs = pool.tile([2, B], F32, tag="mmrhs", name="mmrhs")
    nc.sync.dma_start(rhs[0:1, :], qrow[0:1, 0:B])
    nc.sync.dma_start(rhs[1:2, :], qrow[0:1, B : 2 * B])
    mm_ps = pspool.tile([P, B], F32, tag="cps", name="mmps")
    nc.tensor.matmul(mm_ps[:], lhsT[:], rhs[:], start=True, stop=True)
    mm = pool.tile([P, B], F16, tag=tag, name=tag)
    nc.vector.tensor_copy(mm[:], mm_ps[:])
    return mm


def build_kernel():
    nc = bacc.Bacc(None, target_bir_lowering=False, debug=False)

    def din(name, shape, dtype=F32):
        return nc.dram_tensor(name, list(shape), dtype, kind="ExternalInput").ap()

    rhs1g0 = din("rhs1g0", (4, 108, 32, B), F16)
    rhs1g1 = din("rhs1g1", (4, 72, 32, B), F16)
    xq = din("xq", (96, B * 32))
    l1g0 = din("l1g0", (108, 128), F16)
    l1g1 = din("l1g1", (72, 128), F16)
    l2 = din("l2", (5, 128, 80), F16)
    l3 = din("l3", (5, 120, 40), F16)
    lo = din("lo", (4, 80, 10), F16)
    s1 = din("s1", (96, 3))
    s2 = din("s2", (64, 16), F16)
    s3 = din("s3", (40, 20), F16)
    hw1 = din("hw1", (16, 75))
    hw2 = din("hw2", (20, 400))
    hw3 = din("hw3", (20, 500))
    a1r = din("a1r", (3, 25))
    a1t = din("a1t", (1, 5))
    a2r = din("a2r", (16, 25))
    a2t = din("a2t", (1, 5))
    a3r = din("a3r", (20, 25))
    a3t = din("a3t", (1, 5))
    c1 = din("c1", (1, 1))
    c2 = din("c2", (1, 1))
    c3 = din("c3", (1, 1))
    b1m = din("b1m", (128, 1))
    b2m = din("b2m", (80, 1))
    b3m = din("b3m", (40, 1))
    bo = din("bo", (10, 1))
    ones_row = din("ones_row", (1, 20))
    ones_col = din("ones_col", (20, 1))
    out = nc.dram_tensor("out", [B, 10], F32, kind="ExternalOutput").ap()

    with tile.TileContext(nc) as tc:
        with (
            tc.tile_pool(name="const", bufs=1) as cpool,
            tc.tile_pool(name="stage", bufs=2) as spool,
            tc.tile_pool(name="big", bufs=1) as bpool,
            tc.tile_pool(name="work", bufs=2) as wpool,
            tc.tile_pool(name="hash", bufs=1) as hpool,
            tc.tile_pool(name="feat", bufs=1) as fpool,
            tc.tile_pool(name="psum", bufs=6, space="PSUM") as pspool,
            tc.tile_pool(name="psacc", bufs=2, space="PSUM") as pacc,
        ):
            def load_const(ap, dtype, tag):
                t = cpool.tile(list(ap.shape), dtype, tag=tag, name=tag)
                nc.sync.dma_start(t[:], ap[:])
                return t

            L10 = load_const(l1g0, F16, "l10")
            L11 = load_const(l1g1, F16, "l11")
            L2 = [load_const(l2[k], F16, f"l2_{k}") for k in range(5)]
            L3 = [load_const(l3[k], F16, f"l3_{k}") for k in range(5)]
            LO = [load_const(lo[k], F16, f"lo_{k}") for k in range(4)]
            S1 = load_const(s1, F32, "s1")
            S2 = load_const(s2, F16, "s2")
            S3 = load_const(s3, F16, "s3")
            B1 = load_const(b1m, F32, "b1")
            B2 = load_const(b2m, F32, "b2")
            B3 = load_const(b3m, F32, "b3")
            BO = load_const(bo, F32, "bo")
            ONR = load_const(ones_row, F32, "onr")
            ONC = load_const(ones_col, F32, "onc")
            C1 = load_const(c1, F32, "c1")
            C2 = load_const(c2, F32, "c2")
            C3 = load_const(c3, F32, "c3")

            def asum(ar_ap, C, tag):
                t = hpool.tile([C, 25], F32, tag=tag + "_in", name=tag + "_in")
                nc.sync.dma_start(t[:], ar_ap[:])
                o = hpool.tile([C, 1], F32, tag=tag, name=tag)
                nc.vector.tensor_reduce(o[:], t[:], AX.X, ALU.add)
                return o

            A1v = asum(a1r, 3, "a1v")
            A2v = asum(a2r, 16, "a2v")
            A3v = asum(a3r, 20, "a3v")

            def tailsum(at_ap, tag):
                t = hpool.tile([1, 5], F32, tag=tag + "_in", name=tag + "_in")
                nc.sync.dma_start(t[:], at_ap[:])
                o = hpool.tile([1, 1], F32, tag=tag, name=tag)
                nc.vector.tensor_reduce(o[:], t[:], AX.X, ALU.add)
                nc.vector.tensor_scalar_mul(o[:], o[:], 0.5)
                return o

            T1 = tailsum(a1t, "t1")
            T2 = tailsum(a2t, "t2")
            T3 = tailsum(a3t, "t3")

            def atailraw(at_ap, tag):
                t = hpool.tile([1, 5], F32, tag=tag, name=tag)
                nc.sync.dma_start(t[:], at_ap[:])
                return t

            A1T = atailraw(a1t, "a1traw")
            A2T = atailraw(a2t, "a2traw")
            A3T = atailraw(a3t, "a3traw")

            def aflat(ar_ap, C, tag):
                t = hpool.tile([1, C * 25], F32, tag=tag, name=tag)
                for r in range(C):
                    nc.sync.dma_start(t[0:1, r * 25 : (r + 1) * 25], ar_ap[r : r + 1, :])
                return t

            A1f = aflat(a1r, 3, "a1f")
            A2f = aflat(a2r, 16, "a2f")
            A3f = aflat(a3r, 20, "a3f")

            HW1 = load_const(hw1, F32, "hw1")
            kh1 = _hash_bits(nc, hpool, pacc, HW1, A1f, A1T, C1, 16, ONR)
            HW2 = load_const(hw2, F32, "hw2")
            kh2 = _hash_bits(nc, hpool, pacc, HW2, A2f, A2T, C2, 20, ONR)
            HW3 = load_const(hw3, F32, "hw3")
            kh3 = _hash_bits(nc, hpool, pacc, HW3, A3f, A3T, C3, 20, ONR)

            # ---- layer-1 query hash from fp32 x (streamed in 8 col chunks)
            cm1_ps = pacc.tile([3, B], F32, tag="accps")
            NBC = 16
            bw = B // NBC  # 32 samples, 1024 cols per chunk
            for bc in range(NBC):
                xt = spool.tile([96, bw * 32], F32, tag="stg_f32")
                nc.sync.dma_start(xt[:], xq[:, bc * bw * 32 : (bc + 1) * bw * 32])
                xv = xt[:].rearrange("p (b x) -> p b x", x=32)
                for xi in range(32):
                    nc.tensor.matmul(
                        cm1_ps[:, bc * bw : (bc + 1) * bw],
                        S1[:],
                        xv[:, :, xi],
                        start=(xi == 0),
                        stop=(xi == 31),
                    )
            cm1 = hpool.tile([3, B], F32, tag="cm1")
            nc.vector.tensor_copy(cm1[:], cm1_ps[:])
            qh1 = _query_bits(nc, hpool, pacc, cm1, A1v, T1, C1, 3, ONC, "qh1")
            mm1 = _mask_map(nc, hpool, pspool, kh1, qh1, 128, "mm1")

            # ---- conv1 -> H1 canonical (4 chunks [64=(oxp,ci16), 20*B])
            H1 = [fpool.tile([64, 16 * B], F16, tag=f"h1_{c}", name=f"h1_{c}") for c in range(4)]
            for c in range(4):
                for e in range(8):  # 4 oy rows -> 2 pooled rows each
                    rg0 = spool.tile([108, 4 * B], F16, tag="stg_rg0")
                    rg1 = spool.tile([72, 4 * B], F16, tag="stg_rg1")
                    nc.sync.dma_start(
                        rg0[:].rearrange("p (y b) -> p y b", y=4),
                        rhs1g0[c, :, e * 4 : (e + 1) * 4, :],
                    )
                    nc.sync.dma_start(
                        rg1[:].rearrange("p (y b) -> p y b", y=4),
                        rhs1g1[c, :, e * 4 : (e + 1) * 4, :],
                    )
                    pp = wpool.tile([128, 2 * B], F16, tag="pp")
                    for oy2 in range(2):
                        ev = []
                        for sub in range(2):
                            oy = oy2 * 2 + sub
                            ps = pspool.tile([128, B], F32, tag="cps")
                            nc.tensor.matmul(
                                ps[:], L10[:], rg0[:, oy * B : (oy + 1) * B],
                                start=True, stop=False,
                            )
                            nc.tensor.matmul(
                                ps[:], L11[:], rg1[:, oy * B : (oy + 1) * B],
                                start=False, stop=True,
                            )
                            a = wpool.tile([128, B], F16, tag="act")
                            nc.scalar.activation(a[:], ps[:], AF.Relu, bias=B1[:])
                            am = wpool.tile([128, B], F16, tag="am")
                            nc.vector.tensor_tensor(am[:], a[:], mm1[:], ALU.mult)
                            ev.append(am)
                        nc.vector.tensor_tensor(
                            pp[:, oy2 * B : (oy2 + 1) * B], ev[0][:], ev[1][:], ALU.max
                        )
                    mv = wpool.tile([64, 2 * B], F16, tag="mv")
                    nc.sync.dma_start(mv[:], pp[64:128, :])
                    oyp0 = e * 2  # H1 col row (unpadded)
                    nc.vector.tensor_tensor(
                        H1[c][:, oyp0 * B : (oyp0 + 2) * B], pp[0:64, :], mv[:], ALU.max
                    )

            # ---- layer-2 query hash
            cm2_ps = pacc.tile([16, B], F32, tag="accps")
            first = True
            for c in range(4):
                for oy in range(16):
                    nc.tensor.matmul(
                        cm2_ps[:],
                        S2[:],
                        H1[c][:, oy * B : (oy + 1) * B],
                        start=first,
                        stop=(c == 3 and oy == 15),
                    )
                    first = False
            cm2 = hpool.tile([16, B], F32, tag="cm2")
            nc.vector.tensor_copy(cm2[:], cm2_ps[:])
            qh2 = _query_bits(nc, hpool, pacc, cm2, A2v, T2, C2, 16, ONC, "qh2")
            mm2 = _mask_map(nc, hpool, pspool, kh2, qh2, 80, "mm2")

            # ---- conv2 -> H2 (4 chunks [40=(oxp,ci20), 12*B])
            H2 = [fpool.tile([40, 8 * B], F16, tag=f"h2_{d}", name=f"h2_{d}") for d in range(4)]
            for d in range(4):
                rhs = bpool.tile([128, 20 * B], F16, tag="bigrhs")
                nc.vector.memset(rhs[:, 0 : 2 * B], 0.0)
                nc.vector.memset(rhs[:, 18 * B : 20 * B], 0.0)
                if d == 0:
                    nc.vector.memset(rhs[0:32, 2 * B : 18 * B], 0.0)
                if d == 3:
                    nc.vector.memset(rhs[96:128, 2 * B : 18 * B], 0.0)
                for cc_ in range(4):
                    px_lo = max(4 * d - 2, 4 * cc_)
                    px_hi = min(4 * d + 5, 4 * cc_ + 3)
                    if px_lo > px_hi:
                        continue
                    wx0 = px_lo - (4 * d - 2)
                    src0 = (px_lo - 4 * cc_) * 16
                    np_ = (px_hi - px_lo + 1) * 16
                    nc.sync.dma_start(
                        rhs[wx0 * 16 : wx0 * 16 + np_, 2 * B : 18 * B],
                        H1[cc_][src0 : src0 + np_, :],
                    )
                for oy2 in range(8):
                    pp = wpool.tile([80, B], F16, tag="pp")
                    ev = []
                    for sub in range(2):
                        oy = oy2 * 2 + sub
                        ps = pspool.tile([80, B], F32, tag="cps")
                        for ky in range(5):
                            nc.tensor.matmul(
                                ps[:],
                                L2[ky][:],
                                rhs[:, (oy + ky) * B : (oy + ky + 1) * B],
                                start=(ky == 0),
                                stop=(ky == 4),
                            )
                        a = wpool.tile([80, B], F16, tag="act")
                        nc.scalar.activation(a[:], ps[:], AF.Relu, bias=B2[:])
                        am = wpool.tile([80, B], F16, tag="am")
                        nc.vector.tensor_tensor(am[:], a[:], mm2[:], ALU.mult)
                        ev.append(am)
                    nc.vector.tensor_tensor(pp[:], ev[0][:], ev[1][:], ALU.max)
                    mv = wpool.tile([40, B], F16, tag="mv")
                    nc.sync.dma_start(mv[:], pp[40:80, :])
                    nc.vector.tensor_tensor(
                        H2[d][:, oy2 * B : (oy2 + 1) * B],
                        pp[0:40, :],
                        mv[:],
                        ALU.max,
                    )

            # ---- layer-3 query hash
            cm3_ps = pacc.tile([20, B], F32, tag="accps")
            first = True
            for d in range(4):
                for oy in range(8):
                    nc.tensor.matmul(
                        cm3_ps[:],
                        S3[:],
                        H2[d][:, oy * B : (oy + 1) * B],
                        start=first,
                        stop=(d == 3 and oy == 7),
                    )
                    first = False
            cm3 = hpool.tile([20, B], F32, tag="cm3")
            nc.vector.tensor_copy(cm3[:], cm3_ps[:])
            qh3 = _query_bits(nc, hpool, pacc, cm3, A3v, T3, C3, 20, ONC, "qh3")
            mm3 = _mask_map(nc, hpool, pspool, kh3, qh3, 40, "mm3")

            # ---- conv3 -> H3 (4 chunks [20, 4*B]) reusing h1 slots
            H3 = [fpool.tile([20, 4 * B], F16, tag=f"h1_{d}", name=f"h3_{d}") for d in range(4)]
            for d in range(4):
                rhs = bpool.tile([128, 12 * B], F16, tag="bigrhs")
                nc.vector.memset(rhs[:, 0 : 2 * B], 0.0)
                nc.vector.memset(rhs[:, 10 * B : 12 * B], 0.0)
                if d == 0:
                    nc.vector.memset(rhs[0:64, 2 * B : 10 * B], 0.0)
                if d == 3:
                    nc.vector.memset(rhs[64:128, 2 * B : 10 * B], 0.0)
                for cc_ in range(4):
                    px_lo = max(2 * d - 2, 2 * cc_)
                    px_hi = min(2 * d + 3, 2 * cc_ + 1)
                    if px_lo > px_hi:
                        continue
                    wx0 = px_lo - (2 * d - 2)
                    src0 = (px_lo - 2 * cc_) * 20
                    np_ = (px_hi - px_lo + 1) * 20
                    nc.sync.dma_start(
                        rhs[wx0 * 20 : wx0 * 20 + np_, 2 * B : 10 * B],
                        H2[cc_][src0 : src0 + np_, :],
                    )
                for oy2 in range(4):
                    pp = wpool.tile([40, B], F16, tag="pp")
                    ev = []
                    for sub in range(2):
                        oy = oy2 * 2 + sub
                        ps = pspool.tile([40, B], F32, tag="cps")
                        for ky in range(5):
                            nc.tensor.matmul(
                                ps[:],
                                L3[ky][:],
                                rhs[0:120, (oy + ky) * B : (oy + ky + 1) * B],
                                start=(ky == 0),
                                stop=(ky == 4),
                            )
                        a = wpool.tile([40, B], F16, tag="act")
                        nc.scalar.activation(a[:], ps[:], AF.Relu, bias=B3[:])
                        am = wpool.tile([40, B], F16, tag="am")
                        nc.vector.tensor_tensor(am[:], a[:], mm3[:], ALU.mult)
                        ev.append(am)
                    nc.vector.tensor_tensor(pp[:], ev[0][:], ev[1][:], ALU.max)
                    mv = wpool.tile([20, B], F16, tag="mv")
                    nc.sync.dma_start(mv[:], pp[20:40, :])
                    nc.vector.tensor_tensor(
                        H3[d][:, oy2 * B : (oy2 + 1) * B], pp[0:20, :], mv[:], ALU.max
                    )

            # ---- FC
            fc_ps = pacc.tile([10, B], F32, tag="accps")
            for d in range(4):
                rf = wpool.tile([80, B], F16, tag="rfc")
                for oyp in range(4):
                    nc.sync.dma_start(
                        rf[oyp * 20 : (oyp + 1) * 20, :],
                        H3[d][:, oyp * B : (oyp + 1) * B],
                    )
                nc.tensor.matmul(
                    fc_ps[:], LO[d][:], rf[:], start=(d == 0), stop=(d == 3)
                )
            ob = wpool.tile([10, B], F32, tag="outb")
            nc.vector.tensor_scalar(ob[:], fc_ps[:], BO[:], None, ALU.add)
            nc.sync.dma_start(out.rearrange("b o -> o b"), ob[:])

    nc.compile()
    return nc


# ---------------------------------------------------------------- entry point
def kernel(**inputs) -> np.ndarray:
    in_maps = _host_prep(inputs)
    if "nc" not in _CACHED:
        _CACHED["nc"] = build_kernel()
    nc = _CACHED["nc"]
    res = run_bass_kernel_spmd(nc, in_maps, core_ids=list(range(NCORES)))
    outs = [res.results[i]["out"] for i in range(NCORES)]
    return np.concatenate(outs, axis=0).astype(np.float32)



# revision 12
# speedup vs baseline: 1.7496x; 1.7496x over previous
"""ALSHConvNet on 8 TRN2 NeuronCores — pure data parallel (batch/8 per core).

Per core (512 samples):
- Convs as fp16 banded matmuls on TensorE, fp32 PSUM accumulation, M-order
  (parity, xpair, ch) with channel count padded to a power-of-two block so
  maxpool-x partners sit exactly 64 partitions apart (legal DVE offset).
- Weight-side hash bits AND the full layer-1 ALSH mask are computed on host;
  layer-2/3 query hashes run on DVE/ACT fully overlapped with conv matmuls.
- Mask applied once per layer on the pooled tensor (mask commutes with
  maxpool since it is a constant 0/1 per (sample, channel)).
- Maxpool: y-pairs = DVE max of the two halves of a 2-bank PSUM activation;
  x-pairs = SBUF->SBUF DMA of the upper partition half + DVE max.
- conv1 rhs = single 36-column im2col group; the ky=3,4 pass reuses the same
  SBUF data at a +3 column offset. y-edges of conv2/conv3 skip pad-ky
  matmuls; x-edges use K-trimmed weight tiles (no memsets).
- Host does layout + hashing of host-known quantities only: sharding,
  im2col, banded weights, masks, and the final [10,B] -> [B,10] transpose.
"""

import sys

for p in ("/opt/trn_rl_repo",):
    if p not in sys.path:
        sys.path.insert(0, p)

import numpy as np

import concourse.bass as bass  # noqa
import concourse.bacc as bacc
import concourse.mybir as mybir
import concourse.tile as tile
from concourse.bass_utils import run_bass_kernel_spmd

F32 = mybir.dt.float32
F16 = mybir.dt.float16
AF = mybir.ActivationFunctionType
ALU = mybir.AluOpType
AX = mybir.AxisListType

NCORES = 8
B = 512
R = 0.2
EPS = 1e-12
M_ALSH = 5

_CACHED = {}


# ---------------------------------------------------------------- host hashing
def _kernel_hash_bits(W, a, c):
    """Weight-side ALSH hash bits, fp32, mirroring reference.alsh_mask."""
    W = W.astype(np.float32)
    a = a.astype(np.float32)
    Cout = W.shape[0]
    Kf = W.reshape(Cout, -1)
    norms = np.linalg.norm(Kf, axis=1)
    Kn = Kf / (np.float32(norms.max()) + np.float32(EPS))
    n = np.linalg.norm(Kn, axis=1, keepdims=True).astype(np.float32)
    powers = np.concatenate(
        [n ** np.float32(2 ** (i + 1)) for i in range(M_ALSH)], axis=1
    ).astype(np.float32)
    P = np.concatenate([Kn, powers], axis=1)
    kh = np.mod(np.floor((P @ a + np.float32(c[0])) / np.float32(R)), 2.0)
    return kh.astype(np.float32)  # [Cout] in {0,1}


def _query_hash_bits_l1(x, a1, c1):
    """Per-sample layer-1 query hash bits, fp32, mirroring reference."""
    x = x.astype(np.float32)
    cm = x.mean(axis=(2, 3))                       # [B, 3]
    q = np.repeat(cm, 25, axis=1)                  # [B, 75]
    qn = q / (np.linalg.norm(q, axis=1, keepdims=True) + np.float32(EPS))
    Qv = np.concatenate(
        [qn, np.full((q.shape[0], M_ALSH), 0.5, np.float32)], axis=1
    )
    qh = np.mod(np.floor((Qv @ a1.astype(np.float32) + np.float32(c1[0])) / np.float32(R)), 2.0)
    return qh.astype(np.float32)  # [B] in {0,1}


# ---------------------------------------------------------------- host layout
def _band_lhsT1(W1):
    """conv1 lhsT: pass1 [108,128] (ky 0-2), pass2 [72,128] (ky 3-4).
    M-order: m = par*64 + oxp*16 + co, oxl = 2*oxp + par, wx = oxl + kx."""
    l0 = np.zeros((108, 128), np.float32)
    l1 = np.zeros((72, 128), np.float32)
    for par in range(2):
        for oxp in range(4):
            for co in range(16):
                m = par * 64 + oxp * 16 + co
                oxl = 2 * oxp + par
                for ky in range(5):
                    for ci in range(3):
                        for kx in range(5):
                            wx = oxl + kx
                            if ky < 3:
                                l0[ky * 36 + ci * 12 + wx, m] = W1[co, ci, ky, kx]
                            else:
                                l1[(ky - 3) * 36 + ci * 12 + wx, m] = W1[co, ci, ky, kx]
    return l0.astype(np.float16), l1.astype(np.float16)


def _band_lhsT2(W2):
    """conv2 lhsT per ky: [128 = wx*16+ci, 128 = par*64 + oxp*32 + co(pad32)]."""
    l = np.zeros((5, 128, 128), np.float32)
    for ky in range(5):
        for par in range(2):
            for oxp in range(2):
                for co in range(20):
                    m = par * 64 + oxp * 32 + co
                    oxl = 2 * oxp + par
                    for ci in range(16):
                        for kx in range(5):
                            l[ky, (oxl + kx) * 16 + ci, m] = W2[co, ci, ky, kx]
    return l.astype(np.float16)


def _band_lhsT3(W3):
    """conv3 lhsT per ky: [120 = wx*20+ci, 128 = par*64 + co(pad64)]."""
    l = np.zeros((5, 120, 128), np.float32)
    for ky in range(5):
        for par in range(2):
            for co in range(20):
                m = par * 64 + co
                for ci in range(20):
                    for kx in range(5):
                        l[ky, (par + kx) * 20 + ci, m] = W3[co, ci, ky, kx]
    return l.astype(np.float16)


def _fc_lhsT(Wo):
    """[64 = co(pad64), 160 = (d*4+oyp)*10 + o]; h flat idx = co*16 + oyp*4 + d."""
    l = np.zeros((64, 160), np.float32)
    for d in range(4):
        for oyp in range(4):
            for co in range(20):
                l[co, (d * 4 + oyp) * 10 : (d * 4 + oyp) * 10 + 10] = Wo[
                    :, co * 16 + oyp * 4 + d
                ]
    return l.astype(np.float16)


def _im2col1(xs):
    """g [4, 108, 36, B]: g[c][dy*36+ci*12+wx, y, b] = xpad[b, ci, y+dy, 8c+wx]."""
    xp = np.zeros((B, 3, 38, 36), np.float16)
    xp[:, :, 2:34, 2:34] = xs.astype(np.float16)
    g = np.empty((4, 108, 36, B), np.float16)
    for c in range(4):
        for dy in range(3):
            blk = xp[:, :, dy : dy + 36, 8 * c : 8 * c + 12]  # [B,3,36,12]
            g[c, dy * 36 : (dy + 1) * 36] = (
                blk.transpose(1, 3, 2, 0).reshape(36, 36, B)
            )
    return g


def _host_prep(inputs):
    x = inputs["x"].astype(np.float32)
    W1 = inputs["W1"].astype(np.float32)
    W2 = inputs["W2"].astype(np.float32)
    W3 = inputs["W3"].astype(np.float32)
    b1 = inputs["b1"].astype(np.float32)
    b2 = inputs["b2"].astype(np.float32)
    b3 = inputs["b3"].astype(np.float32)
    a1 = inputs["a1"].astype(np.float32)
    a2 = inputs["a2"].astype(np.float32)
    a3 = inputs["a3"].astype(np.float32)

    kh1 = _kernel_hash_bits(W1, a1, inputs["c1"])  # [16]
    kh2 = _kernel_hash_bits(W2, a2, inputs["c2"])  # [20]
    kh3 = _kernel_hash_bits(W3, a3, inputs["c3"])  # [20]
    qh1 = _query_hash_bits_l1(x, a1, inputs["c1"])  # [4096]
    m1 = (kh1[None, :] == qh1[:, None]).astype(np.float32)  # [4096, 16]

    l1a, l1b = _band_lhsT1(W1)
    l2 = _band_lhsT2(W2)
    l3 = _band_lhsT3(W3)

    def padco(b, n):
        o = np.zeros(n, np.float32)
        o[: b.shape[0]] = b
        return o

    b2p = padco(b2, 32)
    b3p = padco(b3, 64)
    kh2p = padco(kh2, 32)
    kh2cp = padco(1.0 - kh2, 32)
    kh3p = padco(kh3, 64)
    kh3cp = padco(1.0 - kh3, 64)

    shared = {
        "l1a": l1a,
        "l1b": l1b,
        "l2": l2,                      # [5,128,128]
        "l2e0": np.ascontiguousarray(l2[:, 32:128, :]),  # [5,96,128]
        "l3": l3,                      # [5,120,128]
        "l3e0": np.ascontiguousarray(l3[:, 40:120, :]),  # [5,80,128]
        "lo": _fc_lhsT(inputs["Wo"].astype(np.float32)),  # [64,160]
        "s2b": np.tile(np.eye(16, dtype=np.float16), (8, 1)),  # [128,16]
        "s3b": np.concatenate(
            [
                np.concatenate(
                    [np.eye(20, dtype=np.float16), np.zeros((12, 20), np.float16)],
                    axis=0,
                )
                for _ in range(4)
            ],
            axis=0,
        ),  # [128,20]
        "b1m": np.tile(b1, 8).reshape(128, 1),
        "b2m": np.tile(b2p, 4).reshape(128, 1),
        "b3m": np.tile(b3p, 2).reshape(128, 1),
        "bo": inputs["bo"].reshape(10, 1).astype(np.float32),
        "a2v": a2[:400].reshape(16, 25).sum(axis=1).reshape(16, 1),
        "a3v": a3[:500].reshape(20, 25).sum(axis=1).reshape(20, 1),
        "ones16": np.ones((16, 1), np.float32),
        "ones20": np.ones((20, 1), np.float32),
        "tc2": np.array(
            [[0.5 * a2[400:].sum() + inputs["c2"].astype(np.float32)[0]]], np.float32
        ),
        "tc3": np.array(
            [[0.5 * a3[500:].sum() + inputs["c3"].astype(np.float32)[0]]], np.float32
        ),
        "kh2t": np.tile(kh2p, 4).reshape(1, 128),
        "kh2ct": np.tile(kh2cp, 4).reshape(1, 128),
        "kh3t": np.tile(kh3p, 2).reshape(1, 128),
        "kh3ct": np.tile(kh3cp, 2).reshape(1, 128),
    }
    in_maps = []
    for i in range(NCORES):
        xs = x[i * B : (i + 1) * B]
        m = dict(shared)
        m["g"] = _im2col1(xs)
        # [128 = px8*16+ci, B] mask for H1 tiles (same pattern both tiles)
        m["mm1h"] = np.tile(m1[i * B : (i + 1) * B].T, (8, 1)).astype(np.float16)
        in_maps.append(m)
    return in_maps


# ---------------------------------------------------------------- device build
def _parity_ge1(nc, pool, t_ap, C, outtile):
    """outtile = (floor(t) mod 2) as 0/1 via fp32 magic rounding."""
    MAGIC = 12582912.0  # 1.5 * 2^23
    a = pool.tile([C, t_ap.shape[1]], F32, tag="par_a", name="par_a")
    nc.vector.tensor_scalar(a[:], t_ap, 0.5, -0.5, ALU.mult, ALU.add)
    nc.vector.tensor_scalar_add(a[:], a[:], MAGIC)
    nc.vector.tensor_scalar_add(a[:], a[:], -MAGIC)  # a = floor(t/2)
    u = pool.tile([C, t_ap.shape[1]], F32, tag="par_u", name="par_u")
    nc.vector.scalar_tensor_tensor(u[:], a[:], -2.0, t_ap, ALU.mult, ALU.add)
    nc.vector.tensor_scalar(outtile, u[:], 1.0, None, ALU.is_ge)


def build_kernel():
    nc = bacc.Bacc(None, target_bir_lowering=False, debug=False)

    def din(name, shape, dtype=F32):
        return nc.dram_tensor(name, list(shape), dtype, kind="ExternalInput").ap()

    g_in = din("g", (4, 108, 36, B), F16)
    l1a_in = din("l1a", (108, 128), F16)
    l1b_in = din("l1b", (72, 128), F16)
    l2_in = din("l2", (5, 128, 128), F16)
    l2e0_in = din("l2e0", (5, 96, 128), F16)
    l3_in = din("l3", (5, 120, 128), F16)
    l3e0_in = din("l3e0", (5, 80, 128), F16)
    lo_in = din("lo", (64, 160), F16)
    s2b_in = din("s2b", (128, 16), F16)
    s3b_in = din("s3b", (128, 20), F16)
    b1m_in = din("b1m", (128, 1))
    b2m_in = din("b2m", (128, 1))
    b3m_in = din("b3m", (128, 1))
    bo_in = din("bo", (10, 1))
    a2v_in = din("a2v", (16, 1))
    a3v_in = din("a3v", (20, 1))
    ones16_in = din("ones16", (16, 1))
    ones20_in = din("ones20", (20, 1))
    tc2_in = din("tc2", (1, 1))
    tc3_in = din("tc3", (1, 1))
    kh2t_in = din("kh2t", (1, 128))
    kh2ct_in = din("kh2ct", (1, 128))
    kh3t_in = din("kh3t", (1, 128))
    kh3ct_in = din("kh3ct", (1, 128))
    mm1h_in = din("mm1h", (128, B), F16)
    out = nc.dram_tensor("out", [10, B], F32, kind="ExternalOutput").ap()

    with tile.TileContext(nc) as tc:
        with (
            tc.tile_pool(name="const", bufs=1) as cpool,
            tc.tile_pool(name="g", bufs=2) as gpool,
            tc.tile_pool(name="h", bufs=1) as hpool,
            tc.tile_pool(name="rhs", bufs=2) as rpool,
            tc.tile_pool(name="work", bufs=2) as wpool,
            tc.tile_pool(name="q", bufs=1) as qpool,
            tc.tile_pool(name="cps", bufs=2, space="PSUM") as cps,
            tc.tile_pool(name="pacc", bufs=1, space="PSUM") as pacc,
            tc.tile_pool(name="nps", bufs=1, space="PSUM") as npsp,
            tc.tile_pool(name="mps", bufs=1, space="PSUM") as mpsp,
            tc.tile_pool(name="fps", bufs=1, space="PSUM") as fpsp,
        ):
            def load_const(ap, dtype, tag):
                t = cpool.tile(list(ap.shape), dtype, tag=tag, name=tag)
                nc.sync.dma_start(t[:], ap[:])
                return t

            # earliest-needed consts first; g[0] loads go between them
            L1A = load_const(l1a_in, F16, "l1a")
            L1B = load_const(l1b_in, F16, "l1b")
            B1 = load_const(b1m_in, F32, "b1m")

            def load_g(c, half):
                """Half-chunk: cols y in [0,20) (half 0) or [16,36) (half 1)."""
                t = gpool.tile([108, 20 * B], F16, tag="g", name=f"g{c}_{half}")
                tv = t[:].rearrange("p (y b) -> p y b", y=20)
                yb = 16 * half
                for y0, y1 in ((0, 10), (10, 20)):
                    nc.sync.dma_start(tv[:, y0:y1, :], g_in[c, :, yb + y0 : yb + y1, :])
                return t

            G0 = load_g(0, 0)

            MM1H = load_const(mm1h_in, F16, "mm1h")
            L2 = [load_const(l2_in[k], F16, f"l2_{k}") for k in range(5)]
            L2E0 = [load_const(l2e0_in[k], F16, f"l2e0_{k}") for k in range(5)]
            L3 = [load_const(l3_in[k], F16, f"l3_{k}") for k in range(5)]
            L3E0 = [load_const(l3e0_in[k], F16, f"l3e0_{k}") for k in range(5)]
            LO = load_const(lo_in, F16, "lo")
            S2B = load_const(s2b_in, F16, "s2b")
            S3B = load_const(s3b_in, F16, "s3b")
            B2 = load_const(b2m_in, F32, "b2m")
            B3 = load_const(b3m_in, F32, "b3m")
            BO = load_const(bo_in, F32, "bo")
            A2V = load_const(a2v_in, F32, "a2v")
            A3V = load_const(a3v_in, F32, "a3v")
            ON16 = load_const(ones16_in, F32, "ones16")
            ON20 = load_const(ones20_in, F32, "ones20")
            TC2 = load_const(tc2_in, F32, "tc2")
            TC3 = load_const(tc3_in, F32, "tc3")
            KH2T = load_const(kh2t_in, F32, "kh2t")
            KH2CT = load_const(kh2ct_in, F32, "kh2ct")
            KH3T = load_const(kh3t_in, F32, "kh3t")
            KH3CT = load_const(kh3ct_in, F32, "kh3ct")

            H1 = [
                hpool.tile([128, 16 * B], F16, tag=f"h1_{i}", name=f"h1_{i}")
                for i in range(2)
            ]
            H2 = [
                hpool.tile([128, 8 * B], F16, tag=f"h2_{i}", name=f"h2_{i}")
                for i in range(2)
            ]
            H3 = [
                hpool.tile([64, 4 * B], F16, tag=f"h3_{d}", name=f"h3_{d}")
                for d in range(4)
            ]

            cm2_ps = pacc.tile([16, B], F32, tag="cmps", name="cm2_ps")

            # ---------------- conv1 (+ per-tile mask & cm2 accumulation)
            gt = G0
            for hc in range(8):
                c, half = hc // 2, hc % 2
                if hc < 7:
                    gnext = load_g((hc + 1) // 2, (hc + 1) % 2)
                for e in range(4 * half, 4 * half + 4):
                    pp = wpool.tile([128, 2 * B], F16, tag="pp", name="pp")
                    for oy2 in range(2):
                        oy = 4 * e + 2 * oy2
                        yoff = 16 * half  # tile col = y - yoff
                        ps = cps.tile([128, 2 * B], F32, tag="cps", name="cps")
                        gv = gt[:].rearrange("p (y b) -> p y b", y=20)
                        for sub in range(2):
                            nc.tensor.matmul(
                                ps[:, sub * B : (sub + 1) * B],
                                L1A[:],
                                gv[:, oy + sub - yoff, :],
                                start=True,
                                stop=False,
                            )
                            nc.tensor.matmul(
                                ps[:, sub * B : (sub + 1) * B],
                                L1B[:],
                                gv[0:72, oy + sub + 3 - yoff, :],
                                start=False,
                                stop=True,
                            )
                        a = wpool.tile([128, 2 * B], F16, tag="act", name="act")
                        nc.scalar.activation(a[:], ps[:], AF.Relu, bias=B1[:])
                        nc.vector.tensor_tensor(
                            pp[:, oy2 * B : (oy2 + 1) * B],
                            a[:, 0:B],
                            a[:, B : 2 * B],
                            ALU.max,
                        )
                    mv = wpool.tile([64, 2 * B], F16, tag="mv", name="mv")
                    nc.gpsimd.dma_start(mv[:], pp[64:128, :])
                    # chunk c covers px 4c..4c+3 -> H1[c//2] at offset (c%2)*64
                    nc.vector.tensor_tensor(
                        H1[c // 2][
                            (c % 2) * 64 : (c % 2) * 64 + 64,
                            2 * e * B : (2 * e + 2) * B,
                        ],
                        pp[0:64, :],
                        mv[:],
                        ALU.max,
                    )
                if hc < 7:
                    gt = gnext
                if hc % 4 == 3:
                    i = hc // 4
                    h1v = H1[i][:].rearrange("p (y b) -> p y b", y=16)
                    mb = MM1H[:].broadcast_to((128, B, 16)).rearrange("p b y -> p y b")
                    nc.vector.tensor_tensor(h1v, h1v, mb, ALU.mult)
                    for oy in range(16):
                        nc.tensor.matmul(
                            cm2_ps[:],
                            S2B[:],
                            H1[i][:, oy * B : (oy + 1) * B],
                            start=(i == 0 and oy == 0),
                            stop=(i == 1 and oy == 15),
                            skip_group_check=True,
                        )

            # ---------------- query hash chain (stage A: uses PE right away;
            # stage B's PE ops are emitted a d-iteration later so the DVE/ACT
            # part of the chain hides under conv matmuls without stalling PE)
            def qchain_a(cmps_ap, C, AV, ONESC, TC, lname):
                cmsb = qpool.tile([C, B], F32, tag="q_cmsb", name=f"cmsb{lname}")
                nc.scalar.activation(cmsb[:], cmps_ap, AF.Identity)
                sq = qpool.tile([C, B], F32, tag="q_sq", name=f"sq{lname}")
                nc.scalar.activation(sq[:], cmps_ap, AF.Square)
                num_ps = npsp.tile([1, B], F32, tag="nps", name=f"num{lname}")
                nc.tensor.matmul(num_ps[:], AV[:, 0:1], cmsb[:], start=True, stop=True)
                den_ps = mpsp.tile([1, B], F32, tag="mps", name=f"den{lname}")
                nc.tensor.matmul(den_ps[:], ONESC[:, 0:1], sq[:], start=True, stop=True)
                den = qpool.tile([1, B], F32, tag="q_den", name=f"den{lname}")
                nc.scalar.activation(den[:], den_ps[:], AF.Sqrt, scale=25.0)
                nc.vector.tensor_scalar_add(den[:], den[:], EPS)
                rden = qpool.tile([1, B], F32, tag="q_rden", name=f"rden{lname}")
                nc.vector.reciprocal(rden[:], den[:])
                nums = qpool.tile([1, B], F32, tag="q_nums", name=f"nums{lname}")
                nc.scalar.activation(nums[:], num_ps[:], AF.Identity)
                qv = qpool.tile([1, B], F32, tag="q_qv", name=f"qv{lname}")
                nc.vector.tensor_tensor(qv[:], nums[:], rden[:], ALU.mult)
                nc.vector.tensor_scalar(qv[:], qv[:], TC[0:1, 0:1], 1.0 / R, ALU.add, ALU.mult)
                qh = qpool.tile([1, B], F32, tag="q_qh", name=f"qh{lname}")
                _parity_ge1(nc, qpool, qv[:], 1, qh[:])
                qc = qpool.tile([1, B], F32, tag="q_qc", name=f"qc{lname}")
                nc.vector.tensor_scalar(qc[:], qh[:], -1.0, 1.0, ALU.mult, ALU.add)
                return qh, qc

            def qchain_b(qh, qc, KHT, KHCT, lname):
                map_ps = mpsp.tile([128, B], F32, tag="mps", name=f"map{lname}")
                nc.tensor.matmul(map_ps[:], KHT[0:1, :], qh[:], start=True, stop=False)
                nc.tensor.matmul(map_ps[:], KHCT[0:1, :], qc[:], start=False, stop=True)
                mm = hpool.tile([128, B], F16, tag=f"mm{lname}", name=f"mm{lname}")
                nc.scalar.activation(mm[:], map_ps[:], AF.Identity)
                return mm

            # ---------------- conv2
            cm3_ps = pacc.tile([20, B], F32, tag="cmps", name="cm3_ps")
            qh2, qc2 = qchain_a(cm2_ps[:], 16, A2V, ON16, TC2, "2")
            mm2 = None
            for d in range(4):
                nk = 96 if d in (0, 3) else 128
                rhs = rpool.tile([128, 16 * B], F16, tag="rhs2", name=f"rhs2_{d}")
                if d == 0:
                    nc.gpsimd.dma_start(rhs[0:96, :], H1[0][0:96, :])
                elif d == 1:
                    nc.gpsimd.dma_start(rhs[0:96, :], H1[0][32:128, :])
                    nc.gpsimd.dma_start(rhs[96:128, :], H1[1][0:32, :])
                elif d == 2:
                    nc.gpsimd.dma_start(rhs[0:32, :], H1[0][96:128, :])
                    nc.gpsimd.dma_start(rhs[32:128, :], H1[1][0:96, :])
                else:
                    nc.gpsimd.dma_start(rhs[0:96, :], H1[1][32:128, :])

                def lhs2(ky):
                    if d == 0:
                        return L2E0[ky][:]
                    if d == 3:
                        return L2[ky][0:96, :]
                    return L2[ky][:]

                rv = rhs[0:nk, :].rearrange("p (y b) -> p y b", y=16)
                for oy2 in range(8):
                    ps = cps.tile([128, 2 * B], F32, tag="cps", name="cps")
                    for sub in range(2):
                        oy = 2 * oy2 + sub
                        kys = [k for k in range(5) if 0 <= oy + k - 2 < 16]
                        for j, ky in enumerate(kys):
                            nc.tensor.matmul(
                                ps[:, sub * B : (sub + 1) * B],
                                lhs2(ky),
                                rv[:, oy + ky - 2, :],
                                start=(j == 0),
                                stop=(j == len(kys) - 1),
                            )
                    a = wpool.tile([128, 2 * B], F16, tag="act", name="act")
                    nc.scalar.activation(a[:], ps[:], AF.Relu, bias=B2[:])
                    if oy2 % 2 == 0:
                        pp = wpool.tile([128, 2 * B], F16, tag="pp", name="pp")
                    nc.vector.tensor_tensor(
                        pp[:, (oy2 % 2) * B : (oy2 % 2 + 1) * B],
                        a[:, 0:B],
                        a[:, B : 2 * B],
                        ALU.max,
                    )
                    if oy2 % 2 == 1:
                        mv = wpool.tile([64, 2 * B], F16, tag="mv", name="mv")
                        nc.gpsimd.dma_start(mv[:], pp[64:128, :])
                        j2 = oy2 // 2  # pooled-row pair index
                        nc.vector.tensor_tensor(
                            H2[d // 2][
                                (d % 2) * 64 : (d % 2) * 64 + 64,
                                2 * j2 * B : (2 * j2 + 2) * B,
                            ],
                            pp[0:64, :],
                            mv[:],
                            ALU.max,
                        )
                if d == 0:
                    mm2 = qchain_b(qh2, qc2, KH2T, KH2CT, "2")
                if d % 2 == 1:
                    i = d // 2
                    h2v = H2[i][:].rearrange("p (y b) -> p y b", y=8)
                    mb = mm2[:].broadcast_to((128, B, 8)).rearrange("p b y -> p y b")
                    nc.vector.tensor_tensor(h2v, h2v, mb, ALU.mult)
                    for oy in range(8):
                        nc.tensor.matmul(
                            cm3_ps[:],
                            S3B[:],
                            H2[i][:, oy * B : (oy + 1) * B],
                            start=(i == 0 and oy == 0),
                            stop=(i == 1 and oy == 7),
                            skip_group_check=True,
                        )

            # ---------------- conv3
            fc_ps = fpsp.tile([10, B], F32, tag="fps", name="fc_ps")
            qh3, qc3 = qchain_a(cm3_ps[:], 20, A3V, ON20, TC3, "3")
            mm3 = None
            for d in range(4):
                nk = 80 if d in (0, 3) else 120
                rhs = rpool.tile([128, 16 * B], F16, tag="rhs2", name=f"rhs3_{d}")
                wxs = range(2, 6) if d == 0 else (range(0, 4) if d == 3 else range(6))
                for r, wx in enumerate(wxs):
                    px3 = 2 * d - 2 + wx
                    src = H2[px3 // 4]
                    p0 = (px3 % 4) * 32
                    nc.gpsimd.dma_start(
                        rhs[r * 20 : (r + 1) * 20, 0 : 8 * B], src[p0 : p0 + 20, :]
                    )

                def lhs3(ky):
                    if d == 0:
                        return L3E0[ky][:]
                    if d == 3:
                        return L3[ky][0:80, :]
                    return L3[ky][:]

                rv = rhs[0:nk, 0 : 8 * B].rearrange("p (y b) -> p y b", y=8)
                for oy2 in range(4):
                    ps = cps.tile([128, 2 * B], F32, tag="cps", name="cps")
                    for sub in range(2):
                        oy = 2 * oy2 + sub
                        kys = [k for k in range(5) if 0 <= oy + k - 2 < 8]
                        for j, ky in enumerate(kys):
                            nc.tensor.matmul(
                                ps[:, sub * B : (sub + 1) * B],
                                lhs3(ky),
                                rv[:, oy + ky - 2, :],
                                start=(j == 0),
                                stop=(j == len(kys) - 1),
                            )
                    a = wpool.tile([128, 2 * B], F16, tag="act", name="act")
                    nc.scalar.activation(a[:], ps[:], AF.Relu, bias=B3[:])
                    if oy2 % 2 == 0:
                        pp = wpool.tile([128, 2 * B], F16, tag="pp", name="pp")
                    nc.vector.tensor_tensor(
                        pp[:, (oy2 % 2) * B : (oy2 % 2 + 1) * B],
                        a[:, 0:B],
                        a[:, B : 2 * B],
                        ALU.max,
                    )
                    if oy2 % 2 == 1:
                        mv = wpool.tile([64, 2 * B], F16, tag="mv", name="mv")
                        nc.gpsimd.dma_start(mv[:], pp[64:128, :])
                        j2 = oy2 // 2
                        nc.vector.tensor_tensor(
                            H3[d][:, 2 * j2 * B : (2 * j2 + 2) * B],
                            pp[0:64, :],
                            mv[:],
                            ALU.max,
                        )
                if d == 0:
                    mm3 = qchain_b(qh3, qc3, KH3T, KH3CT, "3")
                # mask + FC accumulation for this d-chunk
                h3v = H3[d][:].rearrange("p (y b) -> p y b", y=4)
                mb = mm3[0:64, :].broadcast_to((64, B, 4)).rearrange("p b y -> p y b")
                nc.vector.tensor_tensor(h3v, h3v, mb, ALU.mult)
                for oyp in range(4):
                    nc.tensor.matmul(
                        fc_ps[:],
                        LO[:, (d * 4 + oyp) * 10 : (d * 4 + oyp) * 10 + 10],
                        H3[d][:, oyp * B : (oyp + 1) * B],
                        start=(d == 0 and oyp == 0),
                        stop=(d == 3 and oyp == 3),
                        skip_group_check=True,
                    )

            ob = qpool.tile([10, B], F32, tag="outb", name="outb")
            nc.scalar.activation(ob[:], fc_ps[:], AF.Identity, bias=BO[:])
            nc.sync.dma_start(out[:], ob[:])

    nc.compile()
    return nc


# ---------------------------------------------------------------- entry point
def kernel(**inputs) -> np.ndarray:
    in_maps = _host_prep(inputs)
    if "nc" not in _CACHED:
        _CACHED["nc"] = build_kernel()
    nc = _CACHED["nc"]
    res = run_bass_kernel_spmd(nc, in_maps, core_ids=list(range(NCORES)))
    outs = [res.results[i]["out"].T for i in range(NCORES)]
    return np.ascontiguousarray(np.concatenate(outs, axis=0)).astype(np.float32)
